# revision 10
# baseline (speedup 1.0000x reference)
import sys, os

sys.path.insert(0, "/opt/trn_rl_repo")
sys.path.insert(0, "/root/.axon_site")
import numpy as np

DIM = 2048
DH = 64
H = 16
HKV = 4
RANK = 8
S = 2048
NCORES = 8
NB = S // 128
NSPAN = 4
SPAN = 512
ND = DIM // 128

_CACHE = {}


def _deint_perm():
    p = np.zeros(DH, np.int64)
    for i in range(DH // 2):
        p[i] = 2 * i
        p[32 + i] = 2 * i + 1
    return p


def _tile128(a):
    n, w = a.shape
    nd = n // 128
    return np.ascontiguousarray(
        a.reshape(nd, 128, w).transpose(1, 0, 2).reshape(128, nd * w))


def _prep(inputs):
    f16 = np.float16
    x = np.asarray(inputs["x"], np.float32)[0]
    perm = _deint_perm()

    xtb = np.ascontiguousarray(
        x.reshape(S, ND, 128).transpose(2, 1, 0).reshape(128, ND * S)).astype(f16)

    wq = np.asarray(inputs["wq"], np.float32)[perm] * 0.125
    wk = np.asarray(inputs["wk"], np.float32)[perm]
    wv = np.asarray(inputs["wv"], np.float32)
    wq_a = np.asarray(inputs["wq_a"], np.float32)
    wk_a = np.asarray(inputs["wk_a"], np.float32)
    wv_a = np.asarray(inputs["wv_a"], np.float32)
    wq_b = np.asarray(inputs["wq_b"], np.float32).reshape(H, DH, RANK)[:, perm, :]
    wk_b = np.asarray(inputs["wk_b"], np.float32).reshape(HKV, DH, RANK)[:, perm, :]
    wv_b = np.asarray(inputs["wv_b"], np.float32).reshape(HKV, DH, RANK)

    w1b = _tile128(np.concatenate([wk, wk_a], 0).T.copy()).astype(f16)
    w2b = _tile128(np.concatenate([wv, wv_a], 0).T.copy()).astype(f16)
    w3b = _tile128(np.concatenate([wq, wq_a], 0).T.copy()).astype(f16)

    def baug(wb, scale, swap):
        nh = wb.shape[0]
        out = np.zeros((nh // 2, 128, 128), np.float32)
        for m in range(nh // 2):
            for hh in range(2):
                h = 2 * m + hh
                for d in range(DH):
                    dd = (d + 32) % DH if swap else d
                    col = 64 * hh + d
                    out[m, dd, col] = 1.0
                    out[m, 64:72, col] = wb[h, dd] * scale
        return np.ascontiguousarray(np.concatenate(list(out), axis=1)).astype(f16)

    kbab = baug(wk_b, 2.0, False)
    kbsb = baug(wk_b, 2.0, True)
    qbab = baug(wq_b, 0.25, False)
    qbsb = baug(wq_b, 0.25, True)
    vbab = baug(wv_b, 2.0, False)

    wo = np.asarray(inputs["wo"], np.float32)
    wo_share = np.asarray(inputs["wo_share"], np.float32)
    wc = wo_share + np.tile(wo, (1, H))
    wctb = _tile128(np.ascontiguousarray(wc.T)).astype(f16)

    fc = np.asarray(inputs["freq_cis"], np.float32)
    cos = fc[:, :, 0].T
    sin = fc[:, :, 1].T
    crep = np.tile(cos, (4, 1)).astype(np.float32)
    sr = np.concatenate([-sin, sin], 0)
    srep = np.tile(sr, (2, 1)).astype(np.float32)

    tri = (np.arange(128)[:, None] <= np.arange(128)[None, :]).astype(f16)
    tri4 = np.ascontiguousarray(np.tile(tri, (1, 4)))

    base = dict(
        xtb=xtb, w1b=w1b, w2b=w2b, w3b=w3b,
        kbab=kbab, kbsb=kbsb, qbab=qbab, qbsb=qbsb, vbab=vbab,
        wctb=wctb, crep=crep, srep=srep, tri4=tri4,
    )
    in_maps = []
    for c in range(NCORES):
        cols = np.r_[c * 128:(c + 1) * 128, (15 - c) * 128:(16 - c) * 128]
        m = dict(base)
        m["crep_q"] = np.ascontiguousarray(crep[:, cols])
        m["srep_q"] = np.ascontiguousarray(srep[:, cols])
        in_maps.append(m)
    return in_maps


def _build_program():
    import concourse.bass as bass
    import concourse.bacc as bacc
    import concourse.mybir as mybir
    from concourse import tile

    f16 = mybir.dt.float16
    f32 = mybir.dt.float32
    AF = mybir.ActivationFunctionType

    nc = bacc.Bacc("TRN2", target_bir_lowering=False)

    def inp(name, shape, dt=f16):
        return nc.dram_tensor(name, list(shape), dt, kind="ExternalInput")

    xtb = inp("xtb", (128, ND * S))
    w1b = inp("w1b", (128, ND * 72))
    w2b = inp("w2b", (128, ND * 72))
    w3b = inp("w3b", (128, ND * 72))
    kbab = inp("kbab", (128, 256))
    kbsb = inp("kbsb", (128, 256))
    qbab = inp("qbab", (128, 1024))
    qbsb = inp("qbsb", (128, 1024))
    vbab = inp("vbab", (128, 256))
    wctb = inp("wctb", (128, 8 * DIM))
    crep = inp("crep", (128, S), f32)
    srep = inp("srep", (128, S), f32)
    crep_q = inp("crep_q", (128, 256), f32)
    srep_q = inp("srep_q", (128, 256), f32)
    tri4 = inp("tri4", (128, 512))
    yout = nc.dram_tensor("y", [2, 128, DIM], f32, kind="ExternalOutput")

    pid = nc.partition_id()
    QS = [None]

    with tile.TileContext(nc) as tc:
        qs_list = [None]

        def dq():
            QS[0] = 0 if QS[0] is None else (QS[0] + 1) % 3
            return qs_list[0][QS[0]]

        with (
            tc.tile_pool(name="const", bufs=1) as constp,
            tc.tile_pool(name="xts", bufs=2) as xtp,
            tc.tile_pool(name="pt", bufs=8) as ptp,
            tc.tile_pool(name="ev", bufs=4) as evp,
            tc.tile_pool(name="ps", bufs=2, space="PSUM") as psp,
            tc.tile_pool(name="ps1", bufs=2, space="PSUM") as ps1p,
            tc.tile_pool(name="sc", bufs=2, space="PSUM") as scp,
            tc.tile_pool(name="acc", bufs=2, space="PSUM") as accp,
        ):
            qs_list[0] = [nc.sync, nc.gpsimd, nc.scalar]

            # ---- weights first (stage-1 gate), split for DMA-engine parallelism
            w1s = constp.tile([128, ND, 72], f16, tag="w1s", name="w1s")
            w2s = constp.tile([128, ND, 72], f16, tag="w2s", name="w2s")
            w3s = constp.tile([128, ND, 72], f16, tag="w3s", name="w3s")
            for ws, wb in ((w1s, w1b), (w2s, w2b), (w3s, w3b)):
                for ch in range(4):
                    dq().dma_start(out=ws[:, 4 * ch:4 * ch + 4, :],
                                   in_=wb[:, ch * 288:(ch + 1) * 288].rearrange(
                                       "p (a n) -> p a n", a=4))

            kbas = constp.tile([128, 256], f16, tag="kbas", name="kbas")
            kbss = constp.tile([128, 256], f16, tag="kbss", name="kbss")
            dq().dma_start(out=kbas[:], in_=kbab[:])
            dq().dma_start(out=kbss[:], in_=kbsb[:])
            creps = constp.tile([128, S], f32, tag="creps", name="creps")
            sreps = constp.tile([128, S], f32, tag="sreps", name="sreps")

            ones1 = constp.tile([1, 64], f16, tag="ones1", name="ones1")
            nc.vector.memset(ones1[:], 1.0)
            ck = constp.tile([128, S], f16, tag="ck", name="ck")
            cv = constp.tile([128, S], f16, tag="cv", name="cv")
            cqf = constp.tile([128, NB, 128], f16, tag="cqf", name="cqf")
            cqo = constp.tile([128, 256], f16, tag="cqo", name="cqo")
            nc.vector.memset(ck[:], 0.0)
            nc.vector.memset(cv[:], 0.0)
            nc.vector.memset(cqf[:], 0.0)
            nc.vector.memset(cqo[:], 0.0)

            kT = constp.tile([64, HKV, NB, 128], f16, tag="kT", name="kT")
            vsb = constp.tile([128, NB, HKV, 65], f16, tag="vsb", name="vsb")
            qT = constp.tile([64, 2, H, 128], f16, tag="qT", name="qT")
            onorm = constp.tile([128, 8, 256], f16, tag="onorm", name="onorm")
            nc.vector.memset(vsb[:, :, :, 64], 1.0)

            vbas = constp.tile([128, 256], f16, tag="vbas", name="vbas")
            qbas = constp.tile([128, 1024], f16, tag="qbas", name="qbas")
            qbss = constp.tile([128, 1024], f16, tag="qbss", name="qbss")
            crepq = constp.tile([128, 256], f32, tag="crepq", name="crepq")
            srepq = constp.tile([128, 256], f32, tag="srepq", name="srepq")
            tris = constp.tile([128, 512], f16, tag="tris", name="tris")
            wcts = constp.tile([128, 8, DIM], f16, tag="wcts", name="wcts")

            # ---- stage 1 + k-projection/rope pipelined per span ----
            for sp in range(NSPAN):
                sl = slice(sp * SPAN, (sp + 1) * SPAN)
                xta = xtp.tile([128, ND, SPAN], f16, tag="xta", name="xta")
                for d in range(ND):
                    dq().dma_start(out=xta[:, d, :],
                                   in_=xtb[:, d * S + sp * SPAN: d * S + (sp + 1) * SPAN])
                if sp == 0:
                    # queue rope tables + small late-use consts behind span-0 data
                    for hf in range(4):
                        slh = slice(hf * 512, (hf + 1) * 512)
                        dq().dma_start(out=creps[:, slh], in_=crep[:, slh])
                        dq().dma_start(out=sreps[:, slh], in_=srep[:, slh])
                    dq().dma_start(out=vbas[:], in_=vbab[:])
                    dq().dma_start(out=qbas[:], in_=qbab[:])
                    dq().dma_start(out=qbss[:], in_=qbsb[:])
                    dq().dma_start(out=crepq[:], in_=crep_q[:])
                    dq().dma_start(out=srepq[:], in_=srep_q[:])
                    dq().dma_start(out=tris[:], in_=tri4[:])
                p1 = psp.tile([72, SPAN], f32, tag="ps", name="p1")
                p2 = ps1p.tile([72, SPAN], f32, tag="ps1", name="p2")
                p3 = scp.tile([72, SPAN], f32, tag="sc", name="p3")
                for d in range(ND):
                    nc.tensor.matmul(p1[:], w1s[:, d, :], xta[:, d, :],
                                     start=(d == 0), stop=(d == ND - 1))
                for d in range(ND):
                    nc.tensor.matmul(p2[:], w2s[:, d, :], xta[:, d, :],
                                     start=(d == 0), stop=(d == ND - 1))
                for d in range(ND):
                    nc.tensor.matmul(p3[:], w3s[:, d, :], xta[:, d, :],
                                     start=(d == 0), stop=(d == ND - 1))
                nc.vector.tensor_copy(ck[0:72, sl], p1[:])
                nc.scalar.copy(cv[0:72, sl], p2[:])
                nc.vector.tensor_copy(
                    cqf[0:72, 4 * sp:4 * sp + 4, :],
                    p3[:].rearrange("p (a n) -> p a n", a=4))
                # k B-projection + rope for this span
                for m in range(2):
                    pk = accp.tile([128, SPAN], f32, tag="pacc", name="pk")
                    pks = accp.tile([128, SPAN], f32, tag="pacc", name="pks")
                    nc.tensor.matmul(pk[:], kbas[:, m * 128:(m + 1) * 128], ck[:, sl],
                                     start=True, stop=True)
                    nc.tensor.matmul(pks[:], kbss[:, m * 128:(m + 1) * 128], ck[:, sl],
                                     start=True, stop=True)
                    t1 = evp.tile([128, SPAN], f16, tag="t1", name="t1")
                    t2 = evp.tile([128, SPAN], f16, tag="t2", name="t2")
                    nc.vector.tensor_mul(t1[:], pk[:], creps[:, sl])
                    nc.vector.tensor_mul(t2[:], pks[:], sreps[:, sl])
                    for hh in range(2):
                        kv = 2 * m + hh
                        nc.vector.tensor_add(
                            kT[:, kv, 4 * sp:4 * sp + 4, :],
                            t1[hh * 64:hh * 64 + 64, :].rearrange("p (a n) -> p a n", a=4),
                            t2[hh * 64:hh * 64 + 64, :].rearrange("p (a n) -> p a n", a=4))

            # ---- branch A: select own q columns ----
            for c in range(NCORES):
                with tc.If(pid == c):
                    nc.vector.tensor_copy(
                        cqo[0:72, :],
                        cqf[0:72, c:16 - c:max(15 - 2 * c, 1), :])

            # ---- v + q projections ----
            for t in range(NB):
                pv = psp.tile([128, 4, 64], f32, tag="ps", name="pv")
                nc.tensor.matmul(pv[:], cv[:, t * 128:(t + 1) * 128], vbas[:],
                                 start=True, stop=True)
                nc.scalar.copy(vsb[:, t, :, 0:64], pv[:])
            for m in range(8):
                pq1 = ps1p.tile([128, 256], f32, tag="ps1", name="pq1")
                pq2 = scp.tile([128, 256], f32, tag="sc", name="pq2")
                nc.tensor.matmul(pq1[:], qbas[:, m * 128:(m + 1) * 128], cqo[:],
                                 start=True, stop=True)
                nc.tensor.matmul(pq2[:], qbss[:, m * 128:(m + 1) * 128], cqo[:],
                                 start=True, stop=True)
                t1 = evp.tile([128, 256], f16, tag="t1q", name="t1q")
                t2 = evp.tile([128, 256], f16, tag="t2q", name="t2q")
                nc.vector.tensor_mul(t1[:], pq1[:], crepq[:])
                nc.vector.tensor_mul(t2[:], pq2[:], srepq[:])
                for hh in range(2):
                    h = 2 * m + hh
                    nc.vector.tensor_add(
                        qT[:, :, h, :],
                        t1[hh * 64:hh * 64 + 64, :].rearrange("p (a n) -> p a n", a=2),
                        t2[hh * 64:hh * 64 + 64, :].rearrange("p (a n) -> p a n", a=2))

            # ---- wct prefetch (needed only in stage 4) ----
            wflat = wcts[:].rearrange("p a n -> p (a n)")
            for i in range(16):
                dq().dma_start(out=wflat[:, i * 1024:(i + 1) * 1024],
                               in_=wctb[:, i * 1024:(i + 1) * 1024])

            # ---- stage 3: attention (flat cross-kv pipeline) ----
            def attention(c):
                blocks = [c, 15 - c]
                seq = []
                for kv in range(HKV):
                    for t in range(blocks[1] + 1):
                        if t <= blocks[0]:
                            seq.append((kv, 0, t))
                        seq.append((kv, 1, t))
                last = {}
                for i, (kv, bi, t) in enumerate(seq):
                    last[(kv, bi)] = i
                scring = [(scp, "sc"), (psp, "ps")]
                accring = [(accp, "pacc"), (ps1p, "ps1")]
                scn = [0]
                accn = [0]
                pts = {}
                pacct = {}

                def sctile(shape):
                    pool, tag = scring[scn[0] % 2]
                    scn[0] += 1
                    return pool.tile(shape, f32, tag=tag, name="sct")

                def emit_se(i):
                    kv, bi, t = seq[i]
                    sct = sctile([128, 512])
                    nc.tensor.matmul(
                        sct[:], kT[:, kv, t, :], qT[:, bi, 4 * kv:4 * kv + 4, :],
                        start=True, stop=True)
                    ptt = ptp.tile([128, 512], f16, tag="ptall", name="ptall")
                    nc.scalar.activation(ptt[:], sct[:], AF.Exp)
                    if t == blocks[bi]:
                        nc.vector.tensor_mul(ptt[:], ptt[:], tris[:])
                    pts[i] = ptt

                def emit_pv(i):
                    kv, bi, t = seq[i]
                    if t == 0:
                        pool, tag = accring[accn[0] % 2]
                        accn[0] += 1
                        pacct[(kv, bi)] = pool.tile([65, 512], f32, tag=tag, name="pacc")
                    nc.tensor.matmul(
                        pacct[(kv, bi)][:], vsb[:, t, kv, :], pts.pop(i)[:],
                        start=(t == 0), stop=(t == blocks[bi]))

                def emit_norm(kv, bi):
                    pt = pacct.pop((kv, bi))
                    rs = evp.tile([1, 512], f32, tag="rsum", name="rsum")
                    nc.vector.tensor_copy(rs[:], pt[64:65, :])
                    r32 = evp.tile([1, 512], f32, tag="recs32", name="recs32")
                    nc.vector.reciprocal_approx_fast(out=r32[:], in_=rs[:])
                    r16 = evp.tile([1, 512], f16, tag="recs16", name="recs16")
                    with nc.allow_low_precision(reason="softmax recip bc"):
                        nc.vector.tensor_copy(r16[:], r32[:])
                    rbc = sctile([64, 512])
                    nc.tensor.matmul(rbc[:], ones1[:], r16[:], start=True, stop=True)
                    rbs = evp.tile([64, 512], f16, tag="rbs", name="rbs")
                    with nc.allow_low_precision(reason="softmax recip bc"):
                        nc.vector.tensor_copy(rbs[:], rbc[:])
                    pacc4 = pt[0:64, :].rearrange("p (a n) -> p a n", a=4)
                    rbs4 = rbs.rearrange("p (a n) -> p a n", a=4)
                    for par in range(2):
                        nc.vector.tensor_mul(
                            onorm[64 * par:64 * par + 64, 2 * kv:2 * kv + 2,
                                  bi * 128:(bi + 1) * 128],
                            pacc4[:, par::2, :],
                            rbs4[:, par::2, :])

                LA = 3
                pending = []
                for i in range(min(LA, len(seq))):
                    emit_se(i)
                for i in range(len(seq)):
                    emit_pv(i)
                    if i + LA < len(seq):
                        emit_se(i + LA)
                    kv, bi, t = seq[i]
                    if i == last[(kv, bi)]:
                        pending.append((kv, bi, i))
                    while pending and (i >= pending[0][2] + 3 or i == len(seq) - 1):
                        g = pending.pop(0)
                        emit_norm(g[0], g[1])
                while pending:
                    g = pending.pop(0)
                    emit_norm(g[0], g[1])

            for c in range(NCORES):
                with tc.If(pid == c):
                    attention(c)

            # ---- stage 4: output projection ----
            for e in range(4):
                for bi in range(2):
                    py = psp.tile([128, 512], f32, tag="ps", name="py")
                    for m in range(8):
                        nc.tensor.matmul(
                            py[:], onorm[:, m, bi * 128:(bi + 1) * 128],
                            wcts[:, m, e * 512:(e + 1) * 512],
                            start=(m == 0), stop=(m == 7))
                    ye = evp.tile([128, 512], f32, tag="ye", name="ye")
                    if (e + bi) % 2 == 0:
                        nc.vector.tensor_copy(ye[:], py[:])
                    else:
                        nc.scalar.copy(ye[:], py[:])
                    eng = nc.sync if bi == 0 else nc.gpsimd
                    eng.dma_start(out=yout[bi, :, e * 512:(e + 1) * 512], in_=ye[:])

    nc.finalize()
    return nc


def kernel(**inputs):
    if "nc" not in _CACHE:
        _CACHE["nc"] = _build_program()
    nc = _CACHE["nc"]
    from concourse.bass_utils import run_bass_kernel_spmd

    in_maps = _prep(inputs)
    res = run_bass_kernel_spmd(nc, in_maps, list(range(NCORES)))
    y = np.zeros((1, S, DIM), np.float32)
    for c in range(NCORES):
        yc = res.results[c]["y"]
        y[0, c * 128:(c + 1) * 128] = yc[0]
        y[0, (15 - c) * 128:(16 - c) * 128] = yc[1]
    return y


# revision 11
# speedup vs baseline: 1.1898x; 1.1898x over previous
import sys, os

sys.path.insert(0, "/opt/trn_rl_repo")
sys.path.insert(0, "/root/.axon_site")
import numpy as np

DIM = 2048
DH = 64
H = 16
HKV = 4
RANK = 8
S = 2048
NCORES = 8
NB = S // 128
NSPAN = 4
SPAN = 512
ND = DIM // 128

_CACHE = {}


def _deint_perm():
    p = np.zeros(DH, np.int64)
    for i in range(DH // 2):
        p[i] = 2 * i
        p[32 + i] = 2 * i + 1
    return p


def _tile128(a):
    n, w = a.shape
    nd = n // 128
    return np.ascontiguousarray(
        a.reshape(nd, 128, w).transpose(1, 0, 2).reshape(128, nd * w))


def _prep(inputs):
    f16 = np.float16
    x = np.asarray(inputs["x"], np.float32)[0]
    perm = _deint_perm()

    xtb = np.ascontiguousarray(
        x.reshape(S, ND, 128).transpose(2, 1, 0).reshape(128, ND * S)).astype(f16)

    wq = np.asarray(inputs["wq"], np.float32)[perm] * 0.125
    wk = np.asarray(inputs["wk"], np.float32)[perm]
    wv = np.asarray(inputs["wv"], np.float32)
    wq_a = np.asarray(inputs["wq_a"], np.float32)
    wk_a = np.asarray(inputs["wk_a"], np.float32)
    wv_a = np.asarray(inputs["wv_a"], np.float32)
    wq_b = np.asarray(inputs["wq_b"], np.float32).reshape(H, DH, RANK)[:, perm, :]
    wk_b = np.asarray(inputs["wk_b"], np.float32).reshape(HKV, DH, RANK)[:, perm, :]
    wv_b = np.asarray(inputs["wv_b"], np.float32).reshape(HKV, DH, RANK)

    w1b = _tile128(np.concatenate([wk, wk_a], 0).T.copy()).astype(f16)
    w2b = _tile128(np.concatenate([wv, wv_a], 0).T.copy()).astype(f16)
    w3b = _tile128(np.concatenate([wq, wq_a], 0).T.copy()).astype(f16)

    def baug(wb, scale, swap):
        nh = wb.shape[0]
        out = np.zeros((nh // 2, 128, 128), np.float32)
        for m in range(nh // 2):
            for hh in range(2):
                h = 2 * m + hh
                for d in range(DH):
                    dd = (d + 32) % DH if swap else d
                    col = 64 * hh + d
                    out[m, dd, col] = 1.0
                    out[m, 64:72, col] = wb[h, dd] * scale
        return np.ascontiguousarray(np.concatenate(list(out), axis=1)).astype(f16)

    kbab = baug(wk_b, 2.0, False)
    kbsb = baug(wk_b, 2.0, True)
    qbab = baug(wq_b, 0.25, False)
    qbsb = baug(wq_b, 0.25, True)
    vbab = baug(wv_b, 2.0, False)

    wo = np.asarray(inputs["wo"], np.float32)
    wo_share = np.asarray(inputs["wo_share"], np.float32)
    wc = wo_share + np.tile(wo, (1, H))
    wctb = _tile128(np.ascontiguousarray(wc.T)).astype(f16)

    fc = np.asarray(inputs["freq_cis"], np.float32)
    cos = fc[:, :, 0].T
    sin = fc[:, :, 1].T
    crep = np.tile(cos, (4, 1)).astype(np.float32)
    sr = np.concatenate([-sin, sin], 0)
    srep = np.tile(sr, (2, 1)).astype(np.float32)

    tri = (np.arange(128)[:, None] <= np.arange(128)[None, :]).astype(f16)
    tri4 = np.ascontiguousarray(np.tile(tri, (1, 4)))

    base = dict(
        xtb=xtb, w1b=w1b, w2b=w2b, w3b=w3b,
        kbab=kbab, kbsb=kbsb, qbab=qbab, qbsb=qbsb, vbab=vbab,
        wctb=wctb, crep=crep, srep=srep, tri4=tri4,
    )
    in_maps = []
    for c in range(NCORES):
        cols = np.r_[c * 128:(c + 1) * 128, (15 - c) * 128:(16 - c) * 128]
        m = dict(base)
        m["crep_q"] = np.ascontiguousarray(crep[:, cols])
        m["srep_q"] = np.ascontiguousarray(srep[:, cols])
        in_maps.append(m)
    return in_maps


def _build_program():
    import concourse.bass as bass
    import concourse.bacc as bacc
    import concourse.mybir as mybir
    from concourse import tile

    f16 = mybir.dt.float16
    f32 = mybir.dt.float32
    AF = mybir.ActivationFunctionType

    nc = bacc.Bacc("TRN2", target_bir_lowering=False)

    def inp(name, shape, dt=f16):
        return nc.dram_tensor(name, list(shape), dt, kind="ExternalInput")

    xtb = inp("xtb", (128, ND * S))
    w1b = inp("w1b", (128, ND * 72))
    w2b = inp("w2b", (128, ND * 72))
    w3b = inp("w3b", (128, ND * 72))
    kbab = inp("kbab", (128, 256))
    kbsb = inp("kbsb", (128, 256))
    qbab = inp("qbab", (128, 1024))
    qbsb = inp("qbsb", (128, 1024))
    vbab = inp("vbab", (128, 256))
    wctb = inp("wctb", (128, 8 * DIM))
    crep = inp("crep", (128, S), f32)
    srep = inp("srep", (128, S), f32)
    crep_q = inp("crep_q", (128, 256), f32)
    srep_q = inp("srep_q", (128, 256), f32)
    tri4 = inp("tri4", (128, 512))
    yout = nc.dram_tensor("y", [2, 128, DIM], f32, kind="ExternalOutput")

    pid = nc.partition_id()
    QS = [None]

    with tile.TileContext(nc) as tc:
        qs_list = [None]

        def dq():
            QS[0] = 0 if QS[0] is None else (QS[0] + 1) % 3
            return qs_list[0][QS[0]]

        with (
            tc.tile_pool(name="const", bufs=1) as constp,
            tc.tile_pool(name="xts", bufs=2) as xtp,
            tc.tile_pool(name="pt", bufs=8) as ptp,
            tc.tile_pool(name="ev", bufs=4) as evp,
            tc.tile_pool(name="ps", bufs=2, space="PSUM") as psp,
            tc.tile_pool(name="ps1", bufs=2, space="PSUM") as ps1p,
            tc.tile_pool(name="sc", bufs=2, space="PSUM") as scp,
            tc.tile_pool(name="acc", bufs=2, space="PSUM") as accp,
        ):
            qs_list[0] = [nc.sync, nc.gpsimd, nc.scalar]

            # ---- weights first (stage-1 gate), split for DMA-engine parallelism
            w1s = constp.tile([128, ND, 72], f16, tag="w1s", name="w1s")
            w2s = constp.tile([128, ND, 72], f16, tag="w2s", name="w2s")
            w3s = constp.tile([128, ND, 72], f16, tag="w3s", name="w3s")
            for ws, wb in ((w1s, w1b), (w2s, w2b), (w3s, w3b)):
                for ch in range(4):
                    dq().dma_start(out=ws[:, 4 * ch:4 * ch + 4, :],
                                   in_=wb[:, ch * 288:(ch + 1) * 288].rearrange(
                                       "p (a n) -> p a n", a=4))

            kbas = constp.tile([128, 256], f16, tag="kbas", name="kbas")
            kbss = constp.tile([128, 256], f16, tag="kbss", name="kbss")
            dq().dma_start(out=kbas[:], in_=kbab[:])
            dq().dma_start(out=kbss[:], in_=kbsb[:])
            creps = constp.tile([128, S], f32, tag="creps", name="creps")
            sreps = constp.tile([128, S], f32, tag="sreps", name="sreps")

            ones1 = constp.tile([1, 64], f16, tag="ones1", name="ones1")
            nc.vector.memset(ones1[:], 1.0)
            ck = constp.tile([128, S], f16, tag="ck", name="ck")
            cv = constp.tile([128, S], f16, tag="cv", name="cv")
            cqf = constp.tile([128, NB, 128], f16, tag="cqf", name="cqf")
            cqo = constp.tile([128, 256], f16, tag="cqo", name="cqo")
            nc.vector.memset(ck[:], 0.0)
            nc.vector.memset(cv[:], 0.0)
            nc.vector.memset(cqf[:], 0.0)
            nc.vector.memset(cqo[:], 0.0)

            kT = constp.tile([64, HKV, NB, 128], f16, tag="kT", name="kT")
            vsb = constp.tile([128, NB, HKV, 65], f16, tag="vsb", name="vsb")
            qT = constp.tile([64, 2, H, 128], f16, tag="qT", name="qT")
            onorm = constp.tile([128, 8, 256], f16, tag="onorm", name="onorm")
            nc.vector.memset(vsb[:, :, :, 64], 1.0)

            vbas = constp.tile([128, 256], f16, tag="vbas", name="vbas")
            qbas = constp.tile([128, 1024], f16, tag="qbas", name="qbas")
            qbss = constp.tile([128, 1024], f16, tag="qbss", name="qbss")
            crepq = constp.tile([128, 256], f32, tag="crepq", name="crepq")
            srepq = constp.tile([128, 256], f32, tag="srepq", name="srepq")
            tris = constp.tile([128, 512], f16, tag="tris", name="tris")
            wcts = constp.tile([128, 8, DIM], f16, tag="wcts", name="wcts")

            # ---- stage 1 + k-projection/rope pipelined per span ----
            for sp in range(NSPAN):
                sl = slice(sp * SPAN, (sp + 1) * SPAN)
                xta = xtp.tile([128, ND, SPAN], f16, tag="xta", name="xta")
                for d in range(ND):
                    dq().dma_start(out=xta[:, d, :],
                                   in_=xtb[:, d * S + sp * SPAN: d * S + (sp + 1) * SPAN])
                if sp == 0:
                    # queue rope tables + small late-use consts behind span-0 data
                    for hf in range(4):
                        slh = slice(hf * 512, (hf + 1) * 512)
                        dq().dma_start(out=creps[:, slh], in_=crep[:, slh])
                        dq().dma_start(out=sreps[:, slh], in_=srep[:, slh])
                    dq().dma_start(out=vbas[:], in_=vbab[:])
                    dq().dma_start(out=qbas[:], in_=qbab[:])
                    dq().dma_start(out=qbss[:], in_=qbsb[:])
                    dq().dma_start(out=crepq[:], in_=crep_q[:])
                    dq().dma_start(out=srepq[:], in_=srep_q[:])
                    dq().dma_start(out=tris[:], in_=tri4[:])
                p1 = psp.tile([72, SPAN], f32, tag="ps", name="p1")
                p2 = ps1p.tile([72, SPAN], f32, tag="ps1", name="p2")
                p3 = scp.tile([72, SPAN], f32, tag="sc", name="p3")
                for d in range(ND):
                    nc.tensor.matmul(p1[:], w1s[:, d, :], xta[:, d, :],
                                     start=(d == 0), stop=(d == ND - 1))
                for d in range(ND):
                    nc.tensor.matmul(p2[:], w2s[:, d, :], xta[:, d, :],
                                     start=(d == 0), stop=(d == ND - 1))
                for d in range(ND):
                    nc.tensor.matmul(p3[:], w3s[:, d, :], xta[:, d, :],
                                     start=(d == 0), stop=(d == ND - 1))
                nc.vector.tensor_copy(ck[0:72, sl], p1[:])
                nc.scalar.copy(cv[0:72, sl], p2[:])
                nc.vector.tensor_copy(
                    cqf[0:72, 4 * sp:4 * sp + 4, :],
                    p3[:].rearrange("p (a n) -> p a n", a=4))
                # k B-projection + rope for this span
                for m in range(2):
                    pk = accp.tile([128, SPAN], f32, tag="pacc", name="pk")
                    pks = accp.tile([128, SPAN], f32, tag="pacc", name="pks")
                    nc.tensor.matmul(pk[:], kbas[:, m * 128:(m + 1) * 128], ck[:, sl],
                                     start=True, stop=True)
                    nc.tensor.matmul(pks[:], kbss[:, m * 128:(m + 1) * 128], ck[:, sl],
                                     start=True, stop=True)
                    t1 = evp.tile([128, SPAN], f16, tag="t1", name="t1")
                    t2 = evp.tile([128, SPAN], f16, tag="t2", name="t2")
                    nc.vector.tensor_mul(t1[:], pk[:], creps[:, sl])
                    nc.vector.tensor_mul(t2[:], pks[:], sreps[:, sl])
                    for hh in range(2):
                        kv = 2 * m + hh
                        nc.vector.tensor_add(
                            kT[:, kv, 4 * sp:4 * sp + 4, :],
                            t1[hh * 64:hh * 64 + 64, :].rearrange("p (a n) -> p a n", a=4),
                            t2[hh * 64:hh * 64 + 64, :].rearrange("p (a n) -> p a n", a=4))

            # ---- branch A: select own q columns ----
            for c in range(NCORES):
                with tc.If(pid == c):
                    nc.vector.tensor_copy(
                        cqo[0:72, :],
                        cqf[0:72, c:16 - c:max(15 - 2 * c, 1), :])

            # ---- v + q projections ----
            for t in range(NB):
                pv = psp.tile([128, 4, 64], f32, tag="ps", name="pv")
                nc.tensor.matmul(pv[:], cv[:, t * 128:(t + 1) * 128], vbas[:],
                                 start=True, stop=True)
                nc.scalar.copy(vsb[:, t, :, 0:64], pv[:])
            for m in range(8):
                pq1 = ps1p.tile([128, 256], f32, tag="ps1", name="pq1")
                pq2 = scp.tile([128, 256], f32, tag="sc", name="pq2")
                nc.tensor.matmul(pq1[:], qbas[:, m * 128:(m + 1) * 128], cqo[:],
                                 start=True, stop=True)
                nc.tensor.matmul(pq2[:], qbss[:, m * 128:(m + 1) * 128], cqo[:],
                                 start=True, stop=True)
                t1 = evp.tile([128, 256], f16, tag="t1q", name="t1q")
                t2 = evp.tile([128, 256], f16, tag="t2q", name="t2q")
                nc.vector.tensor_mul(t1[:], pq1[:], crepq[:])
                nc.vector.tensor_mul(t2[:], pq2[:], srepq[:])
                for hh in range(2):
                    h = 2 * m + hh
                    nc.vector.tensor_add(
                        qT[:, :, h, :],
                        t1[hh * 64:hh * 64 + 64, :].rearrange("p (a n) -> p a n", a=2),
                        t2[hh * 64:hh * 64 + 64, :].rearrange("p (a n) -> p a n", a=2))

            # ---- wct prefetch (needed only in stage 4) ----
            wflat = wcts[:].rearrange("p a n -> p (a n)")
            for i in range(16):
                dq().dma_start(out=wflat[:, i * 1024:(i + 1) * 1024],
                               in_=wctb[:, i * 1024:(i + 1) * 1024])

            # ---- stage 3: attention (per-kv, cross-kv prologue overlap) ----
            def attention(c):
                blocks = [c, 15 - c]
                seqs = []
                for kv in range(HKV):
                    s_ = []
                    for t in range(blocks[1] + 1):
                        if t <= blocks[0]:
                            s_.append((0, t))
                        s_.append((1, t))
                    seqs.append(s_)
                scring = [(scp, "sc"), (psp, "ps"), (ps1p, "ps1")]
                scn = [0]
                pts = {}
                pacct = {}
                LA = 5

                def sctile(shape):
                    pool, tag = scring[scn[0] % 3]
                    scn[0] += 1
                    return pool.tile(shape, f32, tag=tag, name="sct")

                def emit_se(kv, i):
                    bi, t = seqs[kv][i]
                    sct = sctile([128, 512])
                    nc.tensor.matmul(
                        sct[:], kT[:, kv, t, :], qT[:, bi, 4 * kv:4 * kv + 4, :],
                        start=True, stop=True)
                    ptt = ptp.tile([128, 512], f16, tag="ptall", name="ptall")
                    nc.scalar.activation(ptt[:], sct[:], AF.Exp)
                    if t == blocks[bi]:
                        nc.vector.tensor_mul(ptt[:], ptt[:], tris[:])
                    pts[(kv, i)] = ptt

                def emit_pv(kv, i):
                    bi, t = seqs[kv][i]
                    if t == 0:
                        pacct[(kv, bi)] = accp.tile([65, 512], f32, tag="pacc", name="pacc")
                    nc.tensor.matmul(
                        pacct[(kv, bi)][:], vsb[:, t, kv, :], pts.pop((kv, i))[:],
                        start=(t == 0), stop=(t == blocks[bi]))

                def emit_norm(kv, bi):
                    pt = pacct.pop((kv, bi))
                    rs = evp.tile([1, 512], f32, tag="rsum", name="rsum")
                    nc.vector.tensor_copy(rs[:], pt[64:65, :])
                    r32 = evp.tile([1, 512], f32, tag="recs32", name="recs32")
                    nc.vector.reciprocal_approx_fast(out=r32[:], in_=rs[:])
                    r16 = evp.tile([1, 512], f16, tag="recs16", name="recs16")
                    with nc.allow_low_precision(reason="softmax recip bc"):
                        nc.vector.tensor_copy(r16[:], r32[:])
                    rbc = sctile([64, 512])
                    nc.tensor.matmul(rbc[:], ones1[:], r16[:], start=True, stop=True)
                    rbs = evp.tile([64, 512], f16, tag="rbs", name="rbs")
                    with nc.allow_low_precision(reason="softmax recip bc"):
                        nc.vector.tensor_copy(rbs[:], rbc[:])
                    pacc4 = pt[0:64, :].rearrange("p (a n) -> p a n", a=4)
                    rbs4 = rbs.rearrange("p (a n) -> p a n", a=4)
                    for par in range(2):
                        nc.vector.tensor_mul(
                            onorm[64 * par:64 * par + 64, 2 * kv:2 * kv + 2,
                                  bi * 128:(bi + 1) * 128],
                            pacc4[:, par::2, :],
                            rbs4[:, par::2, :])

                for i in range(LA):
                    emit_se(0, i)
                for kv in range(HKV):
                    seq = seqs[kv]
                    n = len(seq)
                    for i in range(n):
                        emit_pv(kv, i)
                        j = i + LA
                        if j < n:
                            emit_se(kv, j)
                        elif kv + 1 < HKV and j - n < LA:
                            emit_se(kv + 1, j - n)
                    emit_norm(kv, 0)
                    emit_norm(kv, 1)

            for c in range(NCORES):
                with tc.If(pid == c):
                    attention(c)

            # ---- stage 4: output projection ----
            for e in range(4):
                for bi in range(2):
                    py = psp.tile([128, 512], f32, tag="ps", name="py")
                    for m in range(8):
                        nc.tensor.matmul(
                            py[:], onorm[:, m, bi * 128:(bi + 1) * 128],
                            wcts[:, m, e * 512:(e + 1) * 512],
                            start=(m == 0), stop=(m == 7))
                    ye = evp.tile([128, 512], f32, tag="ye", name="ye")
                    if (e + bi) % 2 == 0:
                        nc.vector.tensor_copy(ye[:], py[:])
                    else:
                        nc.scalar.copy(ye[:], py[:])
                    eng = nc.sync if bi == 0 else nc.gpsimd
                    eng.dma_start(out=yout[bi, :, e * 512:(e + 1) * 512], in_=ye[:])

    nc.finalize()
    return nc


def kernel(**inputs):
    if "nc" not in _CACHE:
        _CACHE["nc"] = _build_program()
    nc = _CACHE["nc"]
    from concourse.bass_utils import run_bass_kernel_spmd

    in_maps = _prep(inputs)
    res = run_bass_kernel_spmd(nc, in_maps, list(range(NCORES)))
    y = np.zeros((1, S, DIM), np.float32)
    for c in range(NCORES):
        yc = res.results[c]["y"]
        y[0, c * 128:(c + 1) * 128] = yc[0]
        y[0, (15 - c) * 128:(16 - c) * 128] = yc[1]
    return y


# revision 13
# speedup vs baseline: 1.2104x; 1.0173x over previous
import sys, os

sys.path.insert(0, "/opt/trn_rl_repo")
sys.path.insert(0, "/root/.axon_site")
import numpy as np

DIM = 2048
DH = 64
H = 16
HKV = 4
RANK = 8
S = 2048
NCORES = 8
NB = S // 128
NSPAN = 4
SPAN = 512
ND = DIM // 128

_CACHE = {}


def _deint_perm():
    p = np.zeros(DH, np.int64)
    for i in range(DH // 2):
        p[i] = 2 * i
        p[32 + i] = 2 * i + 1
    return p


def _tile128(a):
    n, w = a.shape
    nd = n // 128
    return np.ascontiguousarray(
        a.reshape(nd, 128, w).transpose(1, 0, 2).reshape(128, nd * w))


def _prep(inputs):
    f16 = np.float16
    x = np.asarray(inputs["x"], np.float32)[0]
    perm = _deint_perm()

    xtb = np.ascontiguousarray(
        x.reshape(S, ND, 128).transpose(2, 1, 0).reshape(128, ND * S)).astype(f16)

    wq = np.asarray(inputs["wq"], np.float32)[perm] * 0.125
    wk = np.asarray(inputs["wk"], np.float32)[perm]
    wv = np.asarray(inputs["wv"], np.float32)
    wq_a = np.asarray(inputs["wq_a"], np.float32)
    wk_a = np.asarray(inputs["wk_a"], np.float32)
    wv_a = np.asarray(inputs["wv_a"], np.float32)
    wq_b = np.asarray(inputs["wq_b"], np.float32).reshape(H, DH, RANK)[:, perm, :]
    wk_b = np.asarray(inputs["wk_b"], np.float32).reshape(HKV, DH, RANK)[:, perm, :]
    wv_b = np.asarray(inputs["wv_b"], np.float32).reshape(HKV, DH, RANK)

    w1b = _tile128(np.concatenate([wk, wk_a], 0).T.copy()).astype(f16)
    w2b = _tile128(np.concatenate([wv, wv_a], 0).T.copy()).astype(f16)
    w3b = _tile128(np.concatenate([wq, wq_a], 0).T.copy()).astype(f16)

    def baug(wb, scale, swap):
        nh = wb.shape[0]
        out = np.zeros((nh // 2, 128, 128), np.float32)
        for m in range(nh // 2):
            for hh in range(2):
                h = 2 * m + hh
                for d in range(DH):
                    dd = (d + 32) % DH if swap else d
                    col = 64 * hh + d
                    out[m, dd, col] = 1.0
                    out[m, 64:72, col] = wb[h, dd] * scale
        return np.ascontiguousarray(np.concatenate(list(out), axis=1)).astype(f16)

    kbab = baug(wk_b, 2.0, False)
    kbsb = baug(wk_b, 2.0, True)
    qbab = baug(wq_b, 0.25, False)
    qbsb = baug(wq_b, 0.25, True)
    vbab = baug(wv_b, 2.0, False)

    wo = np.asarray(inputs["wo"], np.float32)
    wo_share = np.asarray(inputs["wo_share"], np.float32)
    wc = wo_share + np.tile(wo, (1, H))
    wctb = _tile128(np.ascontiguousarray(wc.T)).astype(f16)

    fc = np.asarray(inputs["freq_cis"], np.float32)
    cos = fc[:, :, 0].T
    sin = fc[:, :, 1].T
    crep = np.tile(cos, (4, 1)).astype(np.float32)
    sr = np.concatenate([-sin, sin], 0)
    srep = np.tile(sr, (2, 1)).astype(np.float32)

    tri = (np.arange(128)[:, None] <= np.arange(128)[None, :]).astype(f16)
    tri4 = np.ascontiguousarray(np.tile(tri, (1, 4)))

    base = dict(
        xtb=xtb, w1b=w1b, w2b=w2b, w3b=w3b,
        kbab=kbab, kbsb=kbsb, qbab=qbab, qbsb=qbsb, vbab=vbab,
        wctb=wctb, crep=crep, srep=srep, tri4=tri4,
    )
    in_maps = []
    for c in range(NCORES):
        cols = np.r_[c * 128:(c + 1) * 128, (15 - c) * 128:(16 - c) * 128]
        m = dict(base)
        m["crep_q"] = np.ascontiguousarray(crep[:, cols])
        m["srep_q"] = np.ascontiguousarray(srep[:, cols])
        in_maps.append(m)
    return in_maps


def _build_program():
    import concourse.bass as bass
    import concourse.bacc as bacc
    import concourse.mybir as mybir
    from concourse import tile

    f16 = mybir.dt.float16
    f32 = mybir.dt.float32
    AF = mybir.ActivationFunctionType

    nc = bacc.Bacc("TRN2", target_bir_lowering=False)

    def inp(name, shape, dt=f16):
        return nc.dram_tensor(name, list(shape), dt, kind="ExternalInput")

    xtb = inp("xtb", (128, ND * S))
    w1b = inp("w1b", (128, ND * 72))
    w2b = inp("w2b", (128, ND * 72))
    w3b = inp("w3b", (128, ND * 72))
    kbab = inp("kbab", (128, 256))
    kbsb = inp("kbsb", (128, 256))
    qbab = inp("qbab", (128, 1024))
    qbsb = inp("qbsb", (128, 1024))
    vbab = inp("vbab", (128, 256))
    wctb = inp("wctb", (128, 8 * DIM))
    crep = inp("crep", (128, S), f32)
    srep = inp("srep", (128, S), f32)
    crep_q = inp("crep_q", (128, 256), f32)
    srep_q = inp("srep_q", (128, 256), f32)
    tri4 = inp("tri4", (128, 512))
    yout = nc.dram_tensor("y", [2, 128, DIM], f32, kind="ExternalOutput")

    pid = nc.partition_id()
    QS = [None]

    with tile.TileContext(nc) as tc:
        qs_list = [None]

        def dq():
            QS[0] = 0 if QS[0] is None else (QS[0] + 1) % 2
            return qs_list[0][QS[0]]

        with (
            tc.tile_pool(name="const", bufs=1) as constp,
            tc.tile_pool(name="xts", bufs=4) as xtp,
            tc.tile_pool(name="pt", bufs=6) as ptp,
            tc.tile_pool(name="ev", bufs=2) as evp,
            tc.tile_pool(name="ps", bufs=2, space="PSUM") as psp,
            tc.tile_pool(name="ps1", bufs=2, space="PSUM") as ps1p,
            tc.tile_pool(name="sc", bufs=2, space="PSUM") as scp,
            tc.tile_pool(name="acc", bufs=2, space="PSUM") as accp,
        ):
            qs_list[0] = [nc.sync, nc.scalar]

            # ---- weights first (stage-1 gate), split for DMA-engine parallelism
            w1s = constp.tile([128, ND, 72], f16, tag="w1s", name="w1s")
            w2s = constp.tile([128, ND, 72], f16, tag="w2s", name="w2s")
            w3s = constp.tile([128, ND, 72], f16, tag="w3s", name="w3s")
            for ws, wb in ((w1s, w1b), (w2s, w2b), (w3s, w3b)):
                for ch in range(4):
                    dq().dma_start(out=ws[:, 4 * ch:4 * ch + 4, :],
                                   in_=wb[:, ch * 288:(ch + 1) * 288].rearrange(
                                       "p (a n) -> p a n", a=4))

            kbas = constp.tile([128, 256], f16, tag="kbas", name="kbas")
            kbss = constp.tile([128, 256], f16, tag="kbss", name="kbss")
            dq().dma_start(out=kbas[:], in_=kbab[:])
            dq().dma_start(out=kbss[:], in_=kbsb[:])
            creps = constp.tile([128, S], f32, tag="creps", name="creps")
            sreps = constp.tile([128, S], f32, tag="sreps", name="sreps")

            ones1 = constp.tile([1, 64], f16, tag="ones1", name="ones1")
            nc.vector.memset(ones1[:], 1.0)
            ck = constp.tile([128, S], f16, tag="ck", name="ck")
            cv = constp.tile([128, S], f16, tag="cv", name="cv")
            cqo = constp.tile([128, 256], f16, tag="cqo", name="cqo")
            nc.vector.memset(ck[:], 0.0)
            nc.vector.memset(cv[:], 0.0)
            nc.vector.memset(cqo[:], 0.0)

            kT = constp.tile([64, HKV, NB, 128], f16, tag="kT", name="kT")
            vsb = constp.tile([128, NB, HKV, 65], f16, tag="vsb", name="vsb")
            qT = constp.tile([64, 2, H, 128], f16, tag="qT", name="qT")
            onorm = constp.tile([128, 8, 256], f16, tag="onorm", name="onorm")
            nc.vector.memset(vsb[:, :, :, 64], 1.0)

            vbas = constp.tile([128, 256], f16, tag="vbas", name="vbas")
            qbas = constp.tile([128, 1024], f16, tag="qbas", name="qbas")
            qbss = constp.tile([128, 1024], f16, tag="qbss", name="qbss")
            crepq = constp.tile([128, 256], f32, tag="crepq", name="crepq")
            srepq = constp.tile([128, 256], f32, tag="srepq", name="srepq")
            tris = constp.tile([128, 512], f16, tag="tris", name="tris")
            wcts = constp.tile([128, 8, DIM], f16, tag="wcts", name="wcts")

            # ---- stage 1 + k-projection/rope pipelined per span ----
            xtas = []
            for sp in range(NSPAN):
                sl = slice(sp * SPAN, (sp + 1) * SPAN)
                xta = xtp.tile([128, ND, SPAN], f16, tag="xta", name="xta")
                for d in range(ND):
                    dq().dma_start(out=xta[:, d, :],
                                   in_=xtb[:, d * S + sp * SPAN: d * S + (sp + 1) * SPAN])
                if sp == 0:
                    # queue rope tables + small late-use consts behind span-0 data
                    for hf in range(4):
                        slh = slice(hf * 512, (hf + 1) * 512)
                        dq().dma_start(out=creps[:, slh], in_=crep[:, slh])
                        dq().dma_start(out=sreps[:, slh], in_=srep[:, slh])
                    dq().dma_start(out=vbas[:], in_=vbab[:])
                    dq().dma_start(out=qbas[:], in_=qbab[:])
                    dq().dma_start(out=qbss[:], in_=qbsb[:])
                    dq().dma_start(out=crepq[:], in_=crep_q[:])
                    dq().dma_start(out=srepq[:], in_=srep_q[:])
                    dq().dma_start(out=tris[:], in_=tri4[:])
                xtas.append(xta)
                p1 = psp.tile([72, SPAN], f32, tag="ps", name="p1")
                p2 = ps1p.tile([72, SPAN], f32, tag="ps1", name="p2")
                for d in range(ND):
                    nc.tensor.matmul(p1[:], w1s[:, d, :], xta[:, d, :],
                                     start=(d == 0), stop=(d == ND - 1))
                for d in range(ND):
                    nc.tensor.matmul(p2[:], w2s[:, d, :], xta[:, d, :],
                                     start=(d == 0), stop=(d == ND - 1))
                nc.vector.tensor_copy(ck[0:72, sl], p1[:])
                nc.scalar.copy(cv[0:72, sl], p2[:])
                # k B-projection + rope for this span
                for m in range(2):
                    pk = accp.tile([128, SPAN], f32, tag="pacc", name="pk")
                    pks = accp.tile([128, SPAN], f32, tag="pacc", name="pks")
                    nc.tensor.matmul(pk[:], kbas[:, m * 128:(m + 1) * 128], ck[:, sl],
                                     start=True, stop=True)
                    nc.tensor.matmul(pks[:], kbss[:, m * 128:(m + 1) * 128], ck[:, sl],
                                     start=True, stop=True)
                    t1 = evp.tile([128, SPAN], f16, tag="t1", name="t1")
                    t2 = evp.tile([128, SPAN], f16, tag="t2", name="t2")
                    nc.vector.tensor_mul(t1[:], pk[:], creps[:, sl])
                    nc.vector.tensor_mul(t2[:], pks[:], sreps[:, sl])
                    for hh in range(2):
                        kv = 2 * m + hh
                        nc.vector.tensor_add(
                            kT[:, kv, 4 * sp:4 * sp + 4, :],
                            t1[hh * 64:hh * 64 + 64, :].rearrange("p (a n) -> p a n", a=4),
                            t2[hh * 64:hh * 64 + 64, :].rearrange("p (a n) -> p a n", a=4))

            # ---- branch: q-base projection for own 2 blocks only ----
            for c in range(NCORES):
                with tc.If(pid == c):
                    for bi, b in enumerate([c, 15 - c]):
                        spb = b // 4
                        off = (b % 4) * 128
                        pq = scp.tile([72, 128], f32, tag="sc", name="pqo")
                        for d in range(ND):
                            nc.tensor.matmul(
                                pq[:], w3s[:, d, :],
                                xtas[spb][:, d, off:off + 128],
                                start=(d == 0), stop=(d == ND - 1))
                        nc.vector.tensor_copy(cqo[0:72, bi * 128:(bi + 1) * 128], pq[:])

            # ---- v + q projections ----
            for t in range(NB):
                pv = psp.tile([128, 4, 64], f32, tag="ps", name="pv")
                nc.tensor.matmul(pv[:], cv[:, t * 128:(t + 1) * 128], vbas[:],
                                 start=True, stop=True)
                nc.scalar.copy(vsb[:, t, :, 0:64], pv[:])
            for m in range(8):
                pq1 = ps1p.tile([128, 256], f32, tag="ps1", name="pq1")
                pq2 = scp.tile([128, 256], f32, tag="sc", name="pq2")
                nc.tensor.matmul(pq1[:], qbas[:, m * 128:(m + 1) * 128], cqo[:],
                                 start=True, stop=True)
                nc.tensor.matmul(pq2[:], qbss[:, m * 128:(m + 1) * 128], cqo[:],
                                 start=True, stop=True)
                t1 = evp.tile([128, 256], f16, tag="t1q", name="t1q")
                t2 = evp.tile([128, 256], f16, tag="t2q", name="t2q")
                nc.vector.tensor_mul(t1[:], pq1[:], crepq[:])
                nc.vector.tensor_mul(t2[:], pq2[:], srepq[:])
                for hh in range(2):
                    h = 2 * m + hh
                    nc.vector.tensor_add(
                        qT[:, :, h, :],
                        t1[hh * 64:hh * 64 + 64, :].rearrange("p (a n) -> p a n", a=2),
                        t2[hh * 64:hh * 64 + 64, :].rearrange("p (a n) -> p a n", a=2))

            # ---- wct prefetch (needed only in stage 4) ----
            wflat = wcts[:].rearrange("p a n -> p (a n)")
            for i in range(16):
                dq().dma_start(out=wflat[:, i * 1024:(i + 1) * 1024],
                               in_=wctb[:, i * 1024:(i + 1) * 1024])

            # ---- stage 3: attention (per-kv, cross-kv prologue overlap) ----
            def attention(c):
                blocks = [c, 15 - c]
                seqs = []
                for kv in range(HKV):
                    s_ = []
                    for t in range(blocks[1] + 1):
                        if t <= blocks[0]:
                            s_.append((0, t))
                        s_.append((1, t))
                    seqs.append(s_)
                scring = [(scp, "sc"), (psp, "ps"), (ps1p, "ps1")]
                scn = [0]
                pts = {}
                pacct = {}
                LA = 5

                def sctile(shape):
                    pool, tag = scring[scn[0] % 3]
                    scn[0] += 1
                    return pool.tile(shape, f32, tag=tag, name="sct")

                def emit_se(kv, i):
                    bi, t = seqs[kv][i]
                    sct = sctile([128, 512])
                    nc.tensor.matmul(
                        sct[:], kT[:, kv, t, :], qT[:, bi, 4 * kv:4 * kv + 4, :],
                        start=True, stop=True)
                    ptt = ptp.tile([128, 512], f16, tag="ptall", name="ptall")
                    nc.scalar.activation(ptt[:], sct[:], AF.Exp)
                    if t == blocks[bi]:
                        nc.vector.tensor_mul(ptt[:], ptt[:], tris[:])
                    pts[(kv, i)] = ptt

                def emit_pv(kv, i):
                    bi, t = seqs[kv][i]
                    if t == 0:
                        pacct[(kv, bi)] = accp.tile([65, 512], f32, tag="pacc", name="pacc")
                    nc.tensor.matmul(
                        pacct[(kv, bi)][:], vsb[:, t, kv, :], pts.pop((kv, i))[:],
                        start=(t == 0), stop=(t == blocks[bi]))

                def emit_norm(kv, bi):
                    pt = pacct.pop((kv, bi))
                    rs = evp.tile([1, 512], f32, tag="rsum", name="rsum")
                    nc.vector.tensor_copy(rs[:], pt[64:65, :])
                    r32 = evp.tile([1, 512], f32, tag="recs32", name="recs32")
                    nc.vector.reciprocal_approx_fast(out=r32[:], in_=rs[:])
                    r16 = evp.tile([1, 512], f16, tag="recs16", name="recs16")
                    with nc.allow_low_precision(reason="softmax recip bc"):
                        nc.vector.tensor_copy(r16[:], r32[:])
                    rbc = sctile([64, 512])
                    nc.tensor.matmul(rbc[:], ones1[:], r16[:], start=True, stop=True)
                    rbs = evp.tile([64, 512], f16, tag="rbs", name="rbs")
                    with nc.allow_low_precision(reason="softmax recip bc"):
                        nc.vector.tensor_copy(rbs[:], rbc[:])
                    pacc4 = pt[0:64, :].rearrange("p (a n) -> p a n", a=4)
                    rbs4 = rbs.rearrange("p (a n) -> p a n", a=4)
                    for par in range(2):
                        nc.vector.tensor_mul(
                            onorm[64 * par:64 * par + 64, 2 * kv:2 * kv + 2,
                                  bi * 128:(bi + 1) * 128],
                            pacc4[:, par::2, :],
                            rbs4[:, par::2, :])

                for i in range(LA):
                    emit_se(0, i)
                for kv in range(HKV):
                    seq = seqs[kv]
                    n = len(seq)
                    for i in range(n):
                        emit_pv(kv, i)
                        j = i + LA
                        if j < n:
                            emit_se(kv, j)
                        elif kv + 1 < HKV and j - n < LA:
                            emit_se(kv + 1, j - n)
                    emit_norm(kv, 0)
                    emit_norm(kv, 1)

            for c in range(NCORES):
                with tc.If(pid == c):
                    attention(c)

            # ---- stage 4: output projection ----
            for e in range(4):
                for bi in range(2):
                    py = psp.tile([128, 512], f32, tag="ps", name="py")
                    for m in range(8):
                        nc.tensor.matmul(
                            py[:], onorm[:, m, bi * 128:(bi + 1) * 128],
                            wcts[:, m, e * 512:(e + 1) * 512],
                            start=(m == 0), stop=(m == 7))
                    ye = evp.tile([128, 512], f32, tag="ye", name="ye")
                    if (e + bi) % 2 == 0:
                        nc.vector.tensor_copy(ye[:], py[:])
                    else:
                        nc.scalar.copy(ye[:], py[:])
                    dq().dma_start(out=yout[bi, :, e * 512:(e + 1) * 512], in_=ye[:])

    nc.finalize()
    return nc


def kernel(**inputs):
    if "nc" not in _CACHE:
        _CACHE["nc"] = _build_program()
    nc = _CACHE["nc"]
    from concourse.bass_utils import run_bass_kernel_spmd

    in_maps = _prep(inputs)
    res = run_bass_kernel_spmd(nc, in_maps, list(range(NCORES)))
    y = np.zeros((1, S, DIM), np.float32)
    for c in range(NCORES):
        yc = res.results[c]["y"]
        y[0, c * 128:(c + 1) * 128] = yc[0]
        y[0, (15 - c) * 128:(16 - c) * 128] = yc[1]
    return y
